# revision 45
# baseline (speedup 1.0000x reference)
"""Trainium2 Bass kernel for nn_CausalLTXAttention (sliding-window + sink causal attention).

Sharding: 8 cores = 2 batches x 4 head-groups (4 heads / 256 inner cols each).
Each core computes column-parallel Q/K/V projections for its 256 inner cols over
the FULL sequence (no halo duplication), the rmsnorm sum-of-squares is completed
with a tiny (16KB) AllGather across the 4 cores of each batch, attention runs
banded (window 512 + sink) per head over all 2048 queries, and the output
projection is row-parallel over the core's 256 e-rows.  Partial outputs (bf16)
are summed on the host (plus bo).

Device layout notes:
  - raw q/k are projected in [l,e] layout; interleaved rope (norm weights /
    logit scale / 1/sqrt(dh) folded into host-precomputed cos/sin tables) is
    applied BEFORE the rmsnorm scale (commutes: rope is linear, scale is
    per-row); the scale arrives after the AllGather and is fused with the
    PE-transpose into qT/kT [e,l] tiles.
  - scores are computed transposed: S^T[k,q] in (k-tile x 256-query-chunk)
    pairs; the band structure means only ~6 k-tiles per chunk, with 5 static
    mask tiles (position-independent band offsets).  Softmax denominator is
    obtained by augmenting V with a ones column in the PV matmul; the
    reciprocal row is partition-broadcast on GpSimd.  Sink key 0 is handled
    by dedicated [1, 256] matmuls for chunks beyond its band tile.
  - matmul operands are declared float32r (fast PE mode, 256 moving dim).
"""

from contextlib import ExitStack

import numpy as np
import ml_dtypes

import concourse.bass as bass
import concourse.bacc as bacc
import concourse.mybir as mybir
import concourse.tile as tile
from concourse.bass_utils import run_bass_kernel_spmd
from concourse.masks import make_identity


# ---- problem constants (hardcoded per the harness contract) ----
B, L, D = 2, 2048, 2048
H, DH = 16, 64
INNER = H * DH  # 1024
WINDOW, SINK = 512, 1
EPS = 1e-6
NCORES = 8
NG = 4  # head groups (cores per batch)
EG = INNER // NG  # 256 inner cols per core
HG = H // NG  # 4 heads per core
NLT = L // 128  # 16 l-tiles
ND = D // 128  # 16 contraction d-tiles
NET = EG // 128  # 2 e-tiles per core
CH = 256  # query chunk for attention
NCH = L // CH  # 8
VW = HG * (DH + 1)  # 260: v tiles with a ones column per head
NDC = D // 512  # 4 output d-chunks

F32 = mybir.dt.float32
F32R = mybir.dt.float32r
BF16 = mybir.dt.bfloat16

REPLICA_GROUPS = [[0, 1, 2, 3], [4, 5, 6, 7]]


def chunk_pairs(c):
    """(k-tile, mask_id) pairs covering causal+window band for query chunk c.

    mask_id: None = fully valid tile; 0: j>=p; 1: j>=p+128; 2: j<p+128;
    3: (j<p)|(p==0) [band tile containing the sink]; 4: j<p; 5: p==0
    (dedicated sink tile NLT, whose kT/vA columns replicate key 0).
    (j = query offset in chunk, p = key offset in tile;
    rel = 128*t - 256*c.)
    """
    out = []
    for t in range(max(0, 2 * c - 4), min(NLT - 1, 2 * c + 1) + 1):
        rel = 128 * t - 256 * c
        if rel == 0:
            m = 0
        elif rel == 128:
            m = 1
        elif rel == -384:
            m = 2
        elif rel == -512:
            m = 3 if t == 0 else 4
        else:
            m = None  # -128, -256: fully inside the band
        out.append((t, m))
    if 2 * c - 4 > 0:
        out.append((NLT, 5))  # sink key 0 no longer in any band tile
    return out


def _build(with_bias: bool):
    nc = bacc.Bacc("TRN2", target_bir_lowering=False, debug=False, num_devices=NCORES)

    xT = nc.dram_tensor("xT", [D, L], BF16, kind="ExternalInput")
    wqT = nc.dram_tensor("wqT", [D, EG], BF16, kind="ExternalInput")
    wkT = nc.dram_tensor("wkT", [D, EG], BF16, kind="ExternalInput")
    wvT = nc.dram_tensor("wvT", [D, EG], BF16, kind="ExternalInput")
    woT = nc.dram_tensor("woT", [EG, D], BF16, kind="ExternalInput")
    tabs_d = nc.dram_tensor("tabs", [L, 4, EG], BF16, kind="ExternalInput")
    mskd = nc.dram_tensor("msk", [6, 128, CH], BF16, kind="ExternalInput")
    if with_bias:
        bqr = nc.dram_tensor("bqr", [1, EG], BF16, kind="ExternalInput")
        bkr = nc.dram_tensor("bkr", [1, EG], BF16, kind="ExternalInput")
        bvr = nc.dram_tensor("bvr", [1, EG], BF16, kind="ExternalInput")
    outp = nc.dram_tensor("outp", [L, D], BF16, kind="ExternalOutput")
    ssel = nc.dram_tensor("ssel", [128 * 32], F32, kind="Internal")
    ssag = nc.dram_tensor("ssag", [NG, 128 * 32], F32, kind="Internal")

    # partition-major views for blocked DMA loads
    xTv = xT.ap().rearrange("(t p) l -> p t l", p=128)  # [128, 16, 2048]
    wqv = wqT.ap().rearrange("(t p) e -> p t e", p=128)  # [128, 16, 256]
    wkv = wkT.ap().rearrange("(t p) e -> p t e", p=128)
    wvv = wvT.ap().rearrange("(t p) e -> p t e", p=128)
    wov = woT.ap().rearrange("(t p) d -> p t d", p=128)  # [128, 2, 2048]
    tbv = tabs_d.ap().rearrange("(lt p) f e -> p lt f e", p=128)  # [128, 16, 4, 256]

    with tile.TileContext(nc) as tc, ExitStack() as ctx:
        consts = ctx.enter_context(tc.tile_pool(name="consts", bufs=1))
        big = ctx.enter_context(tc.tile_pool(name="big", bufs=1))

        ident = consts.tile([128, 128], BF16, tag="ident", name="ident")
        make_identity(nc, ident)
        eps_t = consts.tile([128, 1], F32, tag="eps", name="eps")
        nc.vector.memset(eps_t, EPS)
        one_sc = consts.tile([128, 1], F32, tag="one_sc", name="one_sc")
        nc.vector.memset(one_sc, 1.0)
        ones4 = consts.tile([128, HG], F32, tag="ones4", name="ones4")
        nc.vector.memset(ones4, 1.0)
        mk = [consts.tile([128, CH], BF16, tag=f"mk{i}", name=f"mk{i}") for i in range(6)]
        if with_bias:
            ones_row = consts.tile([1, 128], BF16, tag="ones_row", name="ones_row")
            nc.vector.memset(ones_row, 1.0)
            b_rows = {}
            for nm, dram in (("q", bqr), ("k", bkr), ("v", bvr)):
                b_rows[nm] = consts.tile([1, EG], BF16, tag=f"b_{nm}", name=f"b_{nm}")
                nc.sync.dma_start(out=b_rows[nm], in_=dram.ap())

        # persistent tiles
        rq = [big.tile([128, EG], F32, tag=f"rq{i}", name=f"rq{i}") for i in range(NLT)]
        rk = [big.tile([128, EG], F32, tag=f"rk{i}", name=f"rk{i}") for i in range(NLT)]
        # vA[NLT] is the sink tile: zero except row 0 = copy of key 0's row
        vA = [
            big.tile([128, VW], BF16, tag=f"vA{i}", name=f"vA{i}")
            for i in range(NLT + 1)
        ]
        qT = [big.tile([128, L], BF16, tag=f"qT{i}", name=f"qT{i}") for i in range(NET)]
        # kT has an extra tile of columns [L, L+128): col L = copy of key 0,
        # rest zero (masked out; zeros keep exp() of the dead region finite)
        kT = [
            big.tile([128, L + 128], BF16, tag=f"kT{i}", name=f"kT{i}")
            for i in range(NET)
        ]
        aT = [big.tile([128, L], BF16, tag=f"aT{i}", name=f"aT{i}") for i in range(NET)]
        nc.vector.memset(vA[NLT].bitcast(F32), 0.0)
        for et in range(NET):
            nc.vector.memset(kT[et][:, L : L + 128].bitcast(F32), 0.0)
        ss2 = big.tile([128, 2 * NLT], F32, tag="ss2", name="ss2")
        ssg = big.tile([128, NG, 2 * NLT], F32, tag="ssg", name="ssg")
        sst = big.tile([128, 2 * NLT], F32, tag="sst", name="sst")
        rr = big.tile([128, 2 * NLT], F32, tag="rr", name="rr")
        wog = big.tile([128, NET, D], BF16, tag="wog", name="wog")

        # ---- projection-phase pools (released before attention) ----
        pctx = ctx.enter_context(ExitStack())
        wp = pctx.enter_context(tc.tile_pool(name="wp", bufs=1))
        xp = pctx.enter_context(tc.tile_pool(name="xp", bufs=1))
        tabsp = pctx.enter_context(tc.tile_pool(name="tabsp", bufs=1))
        work = pctx.enter_context(tc.tile_pool(name="work", bufs=1))
        psP = pctx.enter_context(tc.tile_pool(name="psP", bufs=1, space="PSUM"))

        def load_xpair(ip):
            xg2 = xp.tile([128, ND, 256], BF16, tag="xg", bufs=3, name="xg")
            nc.sync.dma_start(out=xg2, in_=xTv[:, :, ip * 256 : (ip + 1) * 256])
            return xg2

        def load_tb(lt):
            tb = tabsp.tile([128, 4, EG], BF16, tag="tb", bufs=3, name="tb")
            nc.sync.dma_start(out=tb, in_=tbv[:, lt])
            return tb

        # DMA issue order is SP-queue order: q weights + first x tiles first
        # (halved, so the first 8 contraction matmuls can start ~4us in),
        # the rest behind them.
        wq_g = wp.tile([128, ND, EG], BF16, tag="wq", name="wq")
        xg_first = xp.tile([128, ND, 256], BF16, tag="xg", bufs=3, name="xg")
        wk_g = wp.tile([128, ND, EG], BF16, tag="wk", name="wk")
        wv_g = wp.tile([128, ND, EG], BF16, tag="wv", name="wv")
        nc.sync.dma_start(out=wq_g[:, 0:8], in_=wqv[:, 0:8])
        nc.sync.dma_start(out=xg_first[:, 0:8], in_=xTv[:, 0:8, 0:256])
        nc.sync.dma_start(out=wq_g[:, 8:16], in_=wqv[:, 8:16])
        nc.sync.dma_start(out=xg_first[:, 8:16], in_=xTv[:, 8:16, 0:256])
        nc.sync.dma_start(out=wk_g[:, 0:8], in_=wkv[:, 0:8])
        nc.sync.dma_start(out=wk_g[:, 8:16], in_=wkv[:, 8:16])
        tb_first = load_tb(0)

        def proj_psum(xg2, sub, wg, bias_key):
            ps = psP.tile([128, EG], F32, tag="pp", bufs=8, name="pp")
            for d in range(ND):
                nc.tensor.matmul(
                    ps,
                    lhsT=xg2[:, d, sub * 128 : (sub + 1) * 128],
                    rhs=wg[:, d, :],
                    start=(d == 0),
                    stop=(d == ND - 1 and not with_bias),
                )
            if with_bias:
                nc.tensor.matmul(
                    ps, lhsT=ones_row, rhs=b_rows[bias_key], start=False, stop=True
                )
            return ps

        # ---------------- phase A: raw q/k projections + rope + partial SS ----
        # squares + rope read the projection PSUM directly (no SBUF staging
        # copy): ACT does the square-accumulate, DVE the rope multiplies.
        for ip in range(NLT // 2):
            xg2 = xg_first if ip == 0 else load_xpair(ip)
            if ip == 2:  # v weights not needed until phase B
                nc.sync.dma_start(out=wv_g, in_=wvv)
            for sub in range(2):
                lt = 2 * ip + sub
                tb = tb_first if lt == 0 else load_tb(lt)
                psq = proj_psum(xg2, sub, wq_g, "q")
                psk = proj_psum(xg2, sub, wk_g, "k")
                for ps, dst, ti, ss_col in ((psq, rq[lt], 0, lt), (psk, rk[lt], 2, NLT + lt)):
                    sq = work.tile([128, EG], F32, tag="sq", bufs=2, name="sq")
                    nc.scalar.activation(
                        sq, ps, mybir.ActivationFunctionType.Square,
                        accum_out=ss2[:, ss_col : ss_col + 1],
                    )
                    tch = tb[:, ti]
                    tsh = tb[:, ti + 1]
                    tmp = work.tile([128, EG], F32, tag="ropetmp", bufs=2, name="ropetmp")
                    nc.vector.tensor_mul(tmp[:, 0::2], ps[:, 1::2], tsh[:, 0::2])
                    nc.vector.tensor_mul(tmp[:, 1::2], ps[:, 0::2], tsh[:, 1::2])
                    nc.vector.tensor_mul(dst, ps, tch)
                    nc.vector.tensor_add(dst, dst, tmp)

        # ---- rmsnorm sum-of-squares completion across the 4-core group ----
        # (issued on the DVE queue so the blocking wait doesn't head-of-line
        # block phase B's x loads on SP)
        sselv = ssel.ap().rearrange("(p j) -> p j", p=128)  # [128, 32]
        nc.gpsimd.dma_start(out=sselv, in_=ss2)
        nc.gpsimd.collective_compute(
            kind="AllGather",
            op=mybir.AluOpType.bypass,
            replica_groups=REPLICA_GROUPS,
            ins=[ssel.ap()],
            outs=[ssag.ap()],
        )
        nc.gpsimd.dma_start(out=ssg, in_=ssag.ap().rearrange("g (p j) -> p g j", p=128))

        # weights/masks needed from the attention phase onward
        nc.sync.dma_start(out=wog, in_=wov)
        for i in range(6):
            nc.sync.dma_start(out=mk[i], in_=mskd.ap()[i])

        # ---------------- phase B: v projection (overlaps the AllGather) ----
        # psum->vA copies on ACT (idle during this phase; on DVE they would
        # delay the rs chain).  The rs = 1/sqrt(mean(ss)+eps) block is
        # injected near the END of phase B: by then the AllGather result is
        # long since landed, so neither ACT nor DVE blocks on it, and rr is
        # ready just before the attention transposes need it.
        for ip in range(NLT // 2):
            xg2 = load_xpair(ip)
            if ip == NLT // 2 - 2:
                nc.vector.tensor_add(sst, ssg[:, 0], ssg[:, 1])
                nc.vector.tensor_add(sst, sst, ssg[:, 2])
                nc.vector.tensor_add(sst, sst, ssg[:, 3])
                nc.scalar.activation(
                    rr, sst, mybir.ActivationFunctionType.Sqrt,
                    bias=eps_t, scale=1.0 / INNER,
                )
                nc.vector.reciprocal(rr, rr)
            for sub in range(2):
                lt = 2 * ip + sub
                psv = proj_psum(xg2, sub, wv_g, "v")
                vA_r = vA[lt].rearrange("p (h c) -> p h c", c=DH + 1)
                nc.scalar.copy(vA_r[:, :, 0:DH], psv.rearrange("p (h c) -> p h c", c=DH))
                nc.vector.tensor_scalar_mul(vA_r[:, :, DH], ones4, one_sc)
                if lt == 0:
                    nc.gpsimd.tensor_copy(vA[NLT][0:1, :], vA[0][0:1, :])

        # ---- release projection pools; open attention/output pools ----
        # PSUM banks are 2KB/partition and pool buffers are bank-granular, so
        # [128, 256]-shaped psums are packed two-per-bank: psS banks hold two
        # k-tiles' scores, psO banks hold two heads' PV outputs, and psP2
        # banks serve both the out-projection and (quartered) the transposes.
        pctx.close()
        esp = ctx.enter_context(tc.tile_pool(name="esp", bufs=1))
        awork = ctx.enter_context(tc.tile_pool(name="awork", bufs=1))
        outw = ctx.enter_context(tc.tile_pool(name="outw", bufs=1))
        psS = ctx.enter_context(tc.tile_pool(name="psS", bufs=1, space="PSUM"))
        psO = ctx.enter_context(tc.tile_pool(name="psO", bufs=1, space="PSUM"))
        psT = ctx.enter_context(tc.tile_pool(name="psT", bufs=1, space="PSUM"))
        psP2 = ctx.enter_context(tc.tile_pool(name="psP2", bufs=1, space="PSUM"))

        def scale_transpose(lt):
            """kn/qn scale by rs (to bf16), then 4 bf16 PE transposes through
            one half-bank psum quartet; copies spread across DVE/ACT."""
            ptq = psT.tile([128, 512], BF16, tag="ptq", bufs=1, name="ptq")
            for qi, (src_t, col, dst_tiles) in enumerate(
                ((rk[lt], NLT + lt, kT), (rq[lt], lt, qT))
            ):
                n = awork.tile([128, EG], BF16, tag="qkn", bufs=4, name="qkn")
                nc.vector.tensor_scalar_mul(n, src_t, rr[:, col : col + 1])
                for et in range(NET):
                    q4 = slice((2 * qi + et) * 128, (2 * qi + et + 1) * 128)
                    nc.tensor.transpose(
                        ptq[:, q4], n[:, et * 128 : (et + 1) * 128], ident
                    )
                    dst = dst_tiles[et][:, lt * 128 : (lt + 1) * 128]
                    if et == 0:
                        nc.vector.tensor_copy(dst, ptq[:, q4])
                    else:
                        nc.scalar.copy(dst, ptq[:, q4])
            if lt == 0:
                for et in range(NET):
                    nc.gpsimd.tensor_copy(kT[et][:, L : L + 1], kT[et][:, 0:1])

        def outproj(c):
            for lt in (2 * c, 2 * c + 1):
                for dc in range(NDC):
                    po = psP2.tile([128, 512], F32, tag="po", bufs=2, name="po")
                    for et in range(NET):
                        nc.tensor.matmul(
                            po,
                            lhsT=aT[et][:, lt * 128 : (lt + 1) * 128],
                            rhs=wog[:, et, dc * 512 : (dc + 1) * 512],
                            start=(et == 0),
                            stop=(et == NET - 1),
                        )
                    osb = outw.tile([128, 512], BF16, tag="osb", bufs=4, name="osb")
                    if dc % 2 == 0:  # split psum->sbuf copies across ACT/DVE
                        nc.scalar.copy(osb, po)
                    else:
                        nc.vector.tensor_copy(osb, po)
                    nc.sync.dma_start(
                        out=outp.ap()[lt * 128 : (lt + 1) * 128, dc * 512 : (dc + 1) * 512],
                        in_=osb,
                    )

        # ---------------- attention, pipelined by query chunk ----------------
        # scale+transposes for chunk c+1 are emitted during chunk c so the
        # DVE-scale -> PE-transpose -> DVE-copy chain hides behind attention.
        def scores(cx, h):
            c, pairs, cs, es_tiles, _ = cx
            et, r0 = h // 2, (h % 2) * 64
            for pi in range(0, len(pairs), 4):
                grp = pairs[pi : pi + 4]
                # two PSUM banks, one exp instruction over all quarters
                pss = psS.tile([128, 1024], F32, tag="pss", bufs=2, name="pss")
                esb = esp.tile([128, 1024], BF16, tag="es", bufs=9, name="es")
                for idx, (t, mid) in enumerate(grp):
                    hs = slice(idx * CH, (idx + 1) * CH)
                    nc.tensor.matmul(
                        pss[:, hs],
                        lhsT=kT[et][r0 : r0 + 64, t * 128 : (t + 1) * 128],
                        rhs=qT[et][r0 : r0 + 64, cs],
                        start=True,
                        stop=True,
                        skip_group_check=True,
                    )
                w = len(grp) * CH
                nc.scalar.activation(
                    esb[:, 0:w], pss[:, 0:w], mybir.ActivationFunctionType.Exp
                )
                for idx, (t, mid) in enumerate(grp):
                    hs = slice(idx * CH, (idx + 1) * CH)
                    if mid in (0, 1, 4):  # triangle masks go to the idle Pool
                        nc.gpsimd.tensor_mul(esb[:, hs], esb[:, hs], mk[mid])
                    elif mid is not None:
                        nc.vector.tensor_mul(esb[:, hs], esb[:, hs], mk[mid])
                    es_tiles[(h, t)] = (esb, hs)

        def pv(cx, h):
            c, pairs, cs, es_tiles, pso_banks = cx
            et, r0 = h // 2, (h % 2) * 64
            if h % 2 == 0:
                pso_banks[h // 2] = psO.tile(
                    [DH + 1, 512], F32, tag="pso", bufs=1, name="pso"
                )
            pso = pso_banks[h // 2][:, (h % 2) * CH : (h % 2 + 1) * CH]
            for i, (t, mid) in enumerate(pairs):
                esb, hs = es_tiles[(h, t)]
                nc.tensor.matmul(
                    pso,
                    lhsT=vA[t][:, h * (DH + 1) : (h + 1) * (DH + 1)],
                    rhs=esb[:, hs],
                    start=(i == 0),
                    stop=(i == len(pairs) - 1),
                    skip_group_check=True,
                )
            rec = awork.tile([1, CH], F32, tag="rec", bufs=3, name="rec")
            nc.vector.reciprocal(rec, pso[DH : DH + 1, :])
            rb = awork.tile([64, CH], F32, tag="rb", bufs=3, name="rb")
            nc.gpsimd.partition_broadcast(rb, rec)
            nc.vector.tensor_mul(aT[et][r0 : r0 + 64, cs], pso[0:DH, :], rb)

        def mkcx(c):
            return (c, chunk_pairs(c), slice(c * CH, (c + 1) * CH), {}, {})

        # warm block: chunks 0 and 1 are thin (2 and 4 band tiles), so their
        # score/PV latencies can't hide within the chunk — interleave them.
        for lt in range(4):
            scale_transpose(lt)
        cx0, cx1 = mkcx(0), mkcx(1)
        scores(cx0, 0)
        scores(cx0, 1)
        scores(cx1, 0)
        scores(cx1, 1)
        pv(cx0, 0)
        scores(cx0, 2)
        pv(cx0, 1)
        scores(cx0, 3)
        scores(cx1, 2)
        pv(cx0, 2)
        scores(cx1, 3)
        pv(cx0, 3)
        pv(cx1, 0)
        scale_transpose(4)
        pv(cx1, 1)
        scale_transpose(5)
        pv(cx1, 2)
        pv(cx1, 3)
        outproj(0)

        for c in range(2, NCH):
            if c + 1 < NCH:
                scale_transpose(2 * (c + 1))
                scale_transpose(2 * (c + 1) + 1)
            cx = mkcx(c)
            scores(cx, 0)
            scores(cx, 1)
            pv(cx, 0)
            scores(cx, 2)
            pv(cx, 1)
            scores(cx, 3)
            pv(cx, 2)
            pv(cx, 3)
            outproj(c - 1)
        outproj(NCH - 1)

    nc.compile()
    return nc


_NC_CACHE = {}


def get_nc(with_bias: bool):
    if with_bias not in _NC_CACHE:
        _NC_CACHE[with_bias] = _build(with_bias)
    return _NC_CACHE[with_bias]


def _fold_tables(cosf, sinf, w):
    """Fold rmsnorm weight w (per channel) into interleaved-rope cos/sin tables.

    Kernel computes raw*tc + swap(raw)*ts with swap pairing (odd->even,
    even->odd), so:
      tc[:, e]    = cos[:, e]    * w[e]
      ts[:, 2i]   = -sin[:, 2i]  * w[2i+1]
      ts[:, 2i+1] =  sin[:, 2i+1]* w[2i]
    """
    tc_ = cosf * w[None, :]
    ts_ = np.empty_like(sinf)
    ts_[:, 0::2] = -sinf[:, 0::2] * w[None, 1::2]
    ts_[:, 1::2] = sinf[:, 1::2] * w[None, 0::2]
    return tc_, ts_


def _build_masks():
    p = np.arange(128)[:, None]
    j = np.arange(CH)[None, :]
    m = np.empty((6, 128, CH), np.float32)
    m[0] = j >= p
    m[1] = j >= p + 128
    m[2] = j < p + 128
    m[3] = (j < p) | (p == 0)
    m[4] = j < p
    m[5] = p == 0
    return m.astype(ml_dtypes.bfloat16)


def make_in_maps(inputs):
    f = lambda k: np.asarray(inputs[k], np.float32)
    x = f("x")
    cosf = f("cos")[0]
    sinf = f("sin")[0]
    lls = f("logit_log_scale")[0, :, 0]
    bq, bk, bv = f("bq"), f("bk"), f("bv")
    with_bias = bool(np.any(bq) or np.any(bk) or np.any(bv))

    qtc_f, qts_f = _fold_tables(cosf, sinf, f("qn_w"))
    ktc_f, kts_f = _fold_tables(cosf, sinf, f("kn_w"))
    qs = (lls * (1.0 / np.sqrt(DH)))[:, None].astype(np.float32)
    tabs_full = np.stack(
        [qtc_f * qs, qts_f * qs, ktc_f, kts_f], axis=1
    )  # [L, 4, INNER]

    Wq, Wk, Wv, Wo = f("Wq"), f("Wk"), f("Wv"), f("Wo")
    msk = _build_masks()

    xT_b = [np.ascontiguousarray(x[b].T).astype(ml_dtypes.bfloat16) for b in range(B)]
    per_g = []
    for g in range(NG):
        sl = slice(g * EG, (g + 1) * EG)
        gm = {
            "wqT": np.ascontiguousarray(Wq[sl].T).astype(ml_dtypes.bfloat16),
            "wkT": np.ascontiguousarray(Wk[sl].T).astype(ml_dtypes.bfloat16),
            "wvT": np.ascontiguousarray(Wv[sl].T).astype(ml_dtypes.bfloat16),
            "woT": np.ascontiguousarray(Wo[:, sl].T).astype(ml_dtypes.bfloat16),
            "tabs": np.ascontiguousarray(tabs_full[:, :, sl]).astype(ml_dtypes.bfloat16),
            "msk": msk,
        }
        if with_bias:
            gm["bqr"] = bq[None, sl].astype(ml_dtypes.bfloat16)
            gm["bkr"] = bk[None, sl].astype(ml_dtypes.bfloat16)
            gm["bvr"] = bv[None, sl].astype(ml_dtypes.bfloat16)
        per_g.append(gm)

    ims = []
    for c in range(NCORES):
        b, g = divmod(c, NG)
        ims.append({"xT": xT_b[b], **per_g[g]})
    return ims, with_bias


last_results = None


def kernel(**inputs):
    global last_results
    ims, with_bias = make_in_maps(inputs)
    nc = get_nc(with_bias)
    res = run_bass_kernel_spmd(nc, ims, core_ids=list(range(NCORES)))
    last_results = res
    out = np.zeros((B, L, D), np.float32)
    for c, om in enumerate(res.results):
        out[c // NG] += np.asarray(om["outp"], dtype=np.float32)
    out += np.asarray(inputs["bo"], np.float32)[None, None, :]
    return out


# revision 46
# speedup vs baseline: 1.0176x; 1.0176x over previous
"""Trainium2 Bass kernel for nn_CausalLTXAttention (sliding-window + sink causal attention).

Sharding: 8 cores = 2 batches x 4 head-groups (4 heads / 256 inner cols each).
Each core computes column-parallel Q/K/V projections for its 256 inner cols over
the FULL sequence (no halo duplication), the rmsnorm sum-of-squares is completed
with a tiny (16KB) AllGather across the 4 cores of each batch, attention runs
banded (window 512 + sink) per head over all 2048 queries, and the output
projection is row-parallel over the core's 256 e-rows.  Partial outputs (bf16)
are summed on the host (plus bo).

Device layout notes:
  - raw q/k are projected in [l,e] layout; interleaved rope (norm weights /
    logit scale / 1/sqrt(dh) folded into host-precomputed cos/sin tables) is
    applied BEFORE the rmsnorm scale (commutes: rope is linear, scale is
    per-row); the scale arrives after the AllGather and is fused with the
    PE-transpose into qT/kT [e,l] tiles.
  - scores are computed transposed: S^T[k,q] in (k-tile x 256-query-chunk)
    pairs; the band structure means only ~6 k-tiles per chunk, with 5 static
    mask tiles (position-independent band offsets).  Softmax denominator is
    obtained by augmenting V with a ones column in the PV matmul; the
    reciprocal row is partition-broadcast on GpSimd.  Sink key 0 is handled
    by dedicated [1, 256] matmuls for chunks beyond its band tile.
  - matmul operands are declared float32r (fast PE mode, 256 moving dim).
"""

from contextlib import ExitStack

import numpy as np
import ml_dtypes

import concourse.bass as bass
import concourse.bacc as bacc
import concourse.mybir as mybir
import concourse.tile as tile
from concourse.bass_utils import run_bass_kernel_spmd
from concourse.masks import make_identity


# ---- problem constants (hardcoded per the harness contract) ----
B, L, D = 2, 2048, 2048
H, DH = 16, 64
INNER = H * DH  # 1024
WINDOW, SINK = 512, 1
EPS = 1e-6
NCORES = 8
NG = 4  # head groups (cores per batch)
EG = INNER // NG  # 256 inner cols per core
HG = H // NG  # 4 heads per core
NLT = L // 128  # 16 l-tiles
ND = D // 128  # 16 contraction d-tiles
NET = EG // 128  # 2 e-tiles per core
CH = 256  # query chunk for attention
NCH = L // CH  # 8
VW = HG * (DH + 1)  # 260: v tiles with a ones column per head
NDC = D // 512  # 4 output d-chunks

F32 = mybir.dt.float32
F32R = mybir.dt.float32r
BF16 = mybir.dt.bfloat16

REPLICA_GROUPS = [[0, 1, 2, 3], [4, 5, 6, 7]]


def chunk_pairs(c):
    """(k-tile, mask_id) pairs covering causal+window band for query chunk c.

    mask_id: None = fully valid tile; 0: j>=p; 1: j>=p+128; 2: j<p+128;
    3: (j<p)|(p==0) [band tile containing the sink]; 4: j<p; 5: p==0
    (dedicated sink tile NLT, whose kT/vA columns replicate key 0).
    (j = query offset in chunk, p = key offset in tile;
    rel = 128*t - 256*c.)
    """
    out = []
    for t in range(max(0, 2 * c - 4), min(NLT - 1, 2 * c + 1) + 1):
        rel = 128 * t - 256 * c
        if rel == 0:
            m = 0
        elif rel == 128:
            m = 1
        elif rel == -384:
            m = 2
        elif rel == -512:
            m = 3 if t == 0 else 4
        else:
            m = None  # -128, -256: fully inside the band
        out.append((t, m))
    if 2 * c - 4 > 0:
        out.append((NLT, 5))  # sink key 0 no longer in any band tile
    return out


def _build(with_bias: bool):
    nc = bacc.Bacc("TRN2", target_bir_lowering=False, debug=False, num_devices=NCORES)

    xT = nc.dram_tensor("xT", [D, L], BF16, kind="ExternalInput")
    wqT = nc.dram_tensor("wqT", [D, EG], BF16, kind="ExternalInput")
    wkT = nc.dram_tensor("wkT", [D, EG], BF16, kind="ExternalInput")
    wvT = nc.dram_tensor("wvT", [D, EG], BF16, kind="ExternalInput")
    woT = nc.dram_tensor("woT", [EG, D], BF16, kind="ExternalInput")
    tabs_d = nc.dram_tensor("tabs", [L, 4, EG], BF16, kind="ExternalInput")
    mskd = nc.dram_tensor("msk", [6, 128, CH], BF16, kind="ExternalInput")
    if with_bias:
        bqr = nc.dram_tensor("bqr", [1, EG], BF16, kind="ExternalInput")
        bkr = nc.dram_tensor("bkr", [1, EG], BF16, kind="ExternalInput")
        bvr = nc.dram_tensor("bvr", [1, EG], BF16, kind="ExternalInput")
    outp = nc.dram_tensor("outp", [L, D], BF16, kind="ExternalOutput")
    ssel = nc.dram_tensor("ssel", [128 * 32], F32, kind="Internal")
    ssag = nc.dram_tensor("ssag", [NG, 128 * 32], F32, kind="Internal")

    # partition-major views for blocked DMA loads
    xTv = xT.ap().rearrange("(t p) l -> p t l", p=128)  # [128, 16, 2048]
    wqv = wqT.ap().rearrange("(t p) e -> p t e", p=128)  # [128, 16, 256]
    wkv = wkT.ap().rearrange("(t p) e -> p t e", p=128)
    wvv = wvT.ap().rearrange("(t p) e -> p t e", p=128)
    wov = woT.ap().rearrange("(t p) d -> p t d", p=128)  # [128, 2, 2048]
    tbv = tabs_d.ap().rearrange("(lt p) f e -> p lt f e", p=128)  # [128, 16, 4, 256]

    with tile.TileContext(nc) as tc, ExitStack() as ctx:
        consts = ctx.enter_context(tc.tile_pool(name="consts", bufs=1))
        big = ctx.enter_context(tc.tile_pool(name="big", bufs=1))

        ident = consts.tile([128, 128], F32, tag="ident", name="ident")
        make_identity(nc, ident)
        eps_t = consts.tile([128, 1], F32, tag="eps", name="eps")
        nc.vector.memset(eps_t, EPS)
        one_sc = consts.tile([128, 1], F32, tag="one_sc", name="one_sc")
        nc.vector.memset(one_sc, 1.0)
        ones4 = consts.tile([128, HG], F32, tag="ones4", name="ones4")
        nc.vector.memset(ones4, 1.0)
        mk = [consts.tile([128, CH], BF16, tag=f"mk{i}", name=f"mk{i}") for i in range(6)]
        if with_bias:
            ones_row = consts.tile([1, 128], BF16, tag="ones_row", name="ones_row")
            nc.vector.memset(ones_row, 1.0)
            b_rows = {}
            for nm, dram in (("q", bqr), ("k", bkr), ("v", bvr)):
                b_rows[nm] = consts.tile([1, EG], BF16, tag=f"b_{nm}", name=f"b_{nm}")
                nc.sync.dma_start(out=b_rows[nm], in_=dram.ap())

        # persistent tiles
        rq = [big.tile([128, EG], F32, tag=f"rq{i}", name=f"rq{i}") for i in range(NLT)]
        rk = [big.tile([128, EG], F32, tag=f"rk{i}", name=f"rk{i}") for i in range(NLT)]
        # vA[NLT] is the sink tile: zero except row 0 = copy of key 0's row
        vA = [
            big.tile([128, VW], BF16, tag=f"vA{i}", name=f"vA{i}")
            for i in range(NLT + 1)
        ]
        qT = [big.tile([128, L], F32R, tag=f"qT{i}", name=f"qT{i}") for i in range(NET)]
        # kT has an extra tile of columns [L, L+128): col L = copy of key 0,
        # rest zero (masked out; zeros keep exp() of the dead region finite)
        kT = [
            big.tile([128, L + 128], F32R, tag=f"kT{i}", name=f"kT{i}")
            for i in range(NET)
        ]
        aT = [big.tile([128, L], BF16, tag=f"aT{i}", name=f"aT{i}") for i in range(NET)]
        nc.vector.memset(vA[NLT].bitcast(F32), 0.0)
        for et in range(NET):
            nc.vector.memset(kT[et][:, L : L + 128].bitcast(F32), 0.0)
        ss2 = big.tile([128, 2 * NLT], F32, tag="ss2", name="ss2")
        ssg = big.tile([128, NG, 2 * NLT], F32, tag="ssg", name="ssg")
        sst = big.tile([128, 2 * NLT], F32, tag="sst", name="sst")
        rr = big.tile([128, 2 * NLT], F32, tag="rr", name="rr")
        wog = big.tile([128, NET, D], BF16, tag="wog", name="wog")

        # ---- projection-phase pools (released before attention) ----
        pctx = ctx.enter_context(ExitStack())
        wp = pctx.enter_context(tc.tile_pool(name="wp", bufs=1))
        xp = pctx.enter_context(tc.tile_pool(name="xp", bufs=1))
        tabsp = pctx.enter_context(tc.tile_pool(name="tabsp", bufs=1))
        work = pctx.enter_context(tc.tile_pool(name="work", bufs=1))
        psP = pctx.enter_context(tc.tile_pool(name="psP", bufs=1, space="PSUM"))

        def load_xpair(ip):
            xg2 = xp.tile([128, ND, 256], BF16, tag="xg", bufs=3, name="xg")
            nc.sync.dma_start(out=xg2, in_=xTv[:, :, ip * 256 : (ip + 1) * 256])
            return xg2

        def load_tb(lt):
            tb = tabsp.tile([128, 4, EG], BF16, tag="tb", bufs=3, name="tb")
            nc.sync.dma_start(out=tb, in_=tbv[:, lt])
            return tb

        # DMA issue order is SP-queue order: q weights + first x tiles first
        # (halved, so the first 8 contraction matmuls can start ~4us in),
        # the rest behind them.
        wq_g = wp.tile([128, ND, EG], BF16, tag="wq", name="wq")
        xg_first = xp.tile([128, ND, 256], BF16, tag="xg", bufs=3, name="xg")
        wk_g = wp.tile([128, ND, EG], BF16, tag="wk", name="wk")
        wv_g = wp.tile([128, ND, EG], BF16, tag="wv", name="wv")
        nc.sync.dma_start(out=wq_g[:, 0:8], in_=wqv[:, 0:8])
        nc.sync.dma_start(out=xg_first[:, 0:8], in_=xTv[:, 0:8, 0:256])
        nc.sync.dma_start(out=wq_g[:, 8:16], in_=wqv[:, 8:16])
        nc.sync.dma_start(out=xg_first[:, 8:16], in_=xTv[:, 8:16, 0:256])
        nc.sync.dma_start(out=wk_g[:, 0:8], in_=wkv[:, 0:8])
        nc.sync.dma_start(out=wk_g[:, 8:16], in_=wkv[:, 8:16])
        tb_first = load_tb(0)

        def proj_psum(xg2, sub, wg, bias_key):
            ps = psP.tile([128, EG], F32, tag="pp", bufs=8, name="pp")
            for d in range(ND):
                nc.tensor.matmul(
                    ps,
                    lhsT=xg2[:, d, sub * 128 : (sub + 1) * 128],
                    rhs=wg[:, d, :],
                    start=(d == 0),
                    stop=(d == ND - 1 and not with_bias),
                )
            if with_bias:
                nc.tensor.matmul(
                    ps, lhsT=ones_row, rhs=b_rows[bias_key], start=False, stop=True
                )
            return ps

        # ---------------- phase A: raw q/k projections + rope + partial SS ----
        # squares + rope read the projection PSUM directly (no SBUF staging
        # copy): ACT does the square-accumulate, DVE the rope multiplies.
        for ip in range(NLT // 2):
            xg2 = xg_first if ip == 0 else load_xpair(ip)
            if ip == 2:  # v weights not needed until phase B
                nc.sync.dma_start(out=wv_g, in_=wvv)
            for sub in range(2):
                lt = 2 * ip + sub
                tb = tb_first if lt == 0 else load_tb(lt)
                psq = proj_psum(xg2, sub, wq_g, "q")
                psk = proj_psum(xg2, sub, wk_g, "k")
                for ps, dst, ti, ss_col in ((psq, rq[lt], 0, lt), (psk, rk[lt], 2, NLT + lt)):
                    sq = work.tile([128, EG], F32, tag="sq", bufs=2, name="sq")
                    nc.scalar.activation(
                        sq, ps, mybir.ActivationFunctionType.Square,
                        accum_out=ss2[:, ss_col : ss_col + 1],
                    )
                    tch = tb[:, ti]
                    tsh = tb[:, ti + 1]
                    tmp = work.tile([128, EG], F32, tag="ropetmp", bufs=2, name="ropetmp")
                    nc.vector.tensor_mul(tmp[:, 0::2], ps[:, 1::2], tsh[:, 0::2])
                    nc.vector.tensor_mul(tmp[:, 1::2], ps[:, 0::2], tsh[:, 1::2])
                    nc.vector.tensor_mul(dst, ps, tch)
                    nc.vector.tensor_add(dst, dst, tmp)

        # ---- rmsnorm sum-of-squares completion across the 4-core group ----
        # (issued on the DVE queue so the blocking wait doesn't head-of-line
        # block phase B's x loads on SP)
        sselv = ssel.ap().rearrange("(p j) -> p j", p=128)  # [128, 32]
        nc.gpsimd.dma_start(out=sselv, in_=ss2)
        nc.gpsimd.collective_compute(
            kind="AllGather",
            op=mybir.AluOpType.bypass,
            replica_groups=REPLICA_GROUPS,
            ins=[ssel.ap()],
            outs=[ssag.ap()],
        )
        nc.gpsimd.dma_start(out=ssg, in_=ssag.ap().rearrange("g (p j) -> p g j", p=128))

        # weights/masks needed from the attention phase onward
        nc.sync.dma_start(out=wog, in_=wov)
        for i in range(6):
            nc.sync.dma_start(out=mk[i], in_=mskd.ap()[i])

        # ---------------- phase B: v projection (overlaps the AllGather) ----
        # psum->vA copies on ACT (idle during this phase; on DVE they would
        # delay the rs chain).  The rs = 1/sqrt(mean(ss)+eps) block is
        # injected near the END of phase B: by then the AllGather result is
        # long since landed, so neither ACT nor DVE blocks on it, and rr is
        # ready just before the attention transposes need it.
        for ip in range(NLT // 2):
            xg2 = load_xpair(ip)
            if ip == NLT // 2 - 2:
                nc.vector.tensor_add(sst, ssg[:, 0], ssg[:, 1])
                nc.vector.tensor_add(sst, sst, ssg[:, 2])
                nc.vector.tensor_add(sst, sst, ssg[:, 3])
                nc.scalar.activation(
                    rr, sst, mybir.ActivationFunctionType.Sqrt,
                    bias=eps_t, scale=1.0 / INNER,
                )
                nc.vector.reciprocal(rr, rr)
            for sub in range(2):
                lt = 2 * ip + sub
                psv = proj_psum(xg2, sub, wv_g, "v")
                vA_r = vA[lt].rearrange("p (h c) -> p h c", c=DH + 1)
                nc.scalar.copy(vA_r[:, :, 0:DH], psv.rearrange("p (h c) -> p h c", c=DH))
                nc.vector.tensor_scalar_mul(vA_r[:, :, DH], ones4, one_sc)
                if lt == 0:
                    nc.gpsimd.tensor_copy(vA[NLT][0:1, :], vA[0][0:1, :])

        # ---- release projection pools; open attention/output pools ----
        # PSUM banks are 2KB/partition and pool buffers are bank-granular, so
        # [128, 256]-shaped psums are packed two-per-bank: psS banks hold two
        # k-tiles' scores, psO banks hold two heads' PV outputs, and psP2
        # banks serve both the out-projection and (quartered) the transposes.
        pctx.close()
        esp = ctx.enter_context(tc.tile_pool(name="esp", bufs=1))
        awork = ctx.enter_context(tc.tile_pool(name="awork", bufs=1))
        outw = ctx.enter_context(tc.tile_pool(name="outw", bufs=1))
        psS = ctx.enter_context(tc.tile_pool(name="psS", bufs=1, space="PSUM"))
        psO = ctx.enter_context(tc.tile_pool(name="psO", bufs=1, space="PSUM"))
        psT = ctx.enter_context(tc.tile_pool(name="psT", bufs=1, space="PSUM"))
        psP2 = ctx.enter_context(tc.tile_pool(name="psP2", bufs=1, space="PSUM"))

        def scale_transpose(lt):
            """kn/qn scale by rs, then 4 PE transposes through one psum bank;
            copies spread across DVE/ACT."""
            ptq = psT.tile([128, 512], F32, tag="ptq", bufs=1, name="ptq")
            for qi, (src_t, col, dst_tiles) in enumerate(
                ((rk[lt], NLT + lt, kT), (rq[lt], lt, qT))
            ):
                n = awork.tile([128, EG], F32, tag="qkn", bufs=4, name="qkn")
                nc.vector.tensor_scalar_mul(n, src_t, rr[:, col : col + 1])
                for et in range(NET):
                    q4 = slice((2 * qi + et) * 128, (2 * qi + et + 1) * 128)
                    nc.tensor.transpose(
                        ptq[:, q4], n[:, et * 128 : (et + 1) * 128], ident
                    )
                    dst = dst_tiles[et][:, lt * 128 : (lt + 1) * 128]
                    if et == 0:
                        nc.vector.tensor_copy(dst, ptq[:, q4])
                    else:
                        nc.scalar.copy(dst, ptq[:, q4])
            if lt == 0:
                for et in range(NET):
                    nc.gpsimd.tensor_copy(kT[et][:, L : L + 1], kT[et][:, 0:1])

        def outproj(c):
            for lt in (2 * c, 2 * c + 1):
                for dc in range(NDC):
                    po = psP2.tile([128, 512], F32, tag="po", bufs=2, name="po")
                    for et in range(NET):
                        nc.tensor.matmul(
                            po,
                            lhsT=aT[et][:, lt * 128 : (lt + 1) * 128],
                            rhs=wog[:, et, dc * 512 : (dc + 1) * 512],
                            start=(et == 0),
                            stop=(et == NET - 1),
                        )
                    osb = outw.tile([128, 512], BF16, tag="osb", bufs=4, name="osb")
                    if dc % 2 == 0:  # split psum->sbuf copies across ACT/DVE
                        nc.scalar.copy(osb, po)
                    else:
                        nc.vector.tensor_copy(osb, po)
                    nc.sync.dma_start(
                        out=outp.ap()[lt * 128 : (lt + 1) * 128, dc * 512 : (dc + 1) * 512],
                        in_=osb,
                    )

        # ---------------- attention, pipelined by query chunk ----------------
        # scale+transposes for chunk c+1 are emitted during chunk c so the
        # DVE-scale -> PE-transpose -> DVE-copy chain hides behind attention.
        def scores(cx, h):
            c, pairs, cs, es_tiles, _ = cx
            et, r0 = h // 2, (h % 2) * 64
            for pi in range(0, len(pairs), 4):
                grp = pairs[pi : pi + 4]
                # two PSUM banks, one exp instruction over all quarters
                pss = psS.tile([128, 1024], F32, tag="pss", bufs=2, name="pss")
                esb = esp.tile([128, 1024], BF16, tag="es", bufs=9, name="es")
                for idx, (t, mid) in enumerate(grp):
                    hs = slice(idx * CH, (idx + 1) * CH)
                    nc.tensor.matmul(
                        pss[:, hs],
                        lhsT=kT[et][r0 : r0 + 64, t * 128 : (t + 1) * 128],
                        rhs=qT[et][r0 : r0 + 64, cs],
                        start=True,
                        stop=True,
                        skip_group_check=True,
                    )
                w = len(grp) * CH
                nc.scalar.activation(
                    esb[:, 0:w], pss[:, 0:w], mybir.ActivationFunctionType.Exp
                )
                for idx, (t, mid) in enumerate(grp):
                    hs = slice(idx * CH, (idx + 1) * CH)
                    if mid in (0, 1, 4):  # triangle masks go to the idle Pool
                        nc.gpsimd.tensor_mul(esb[:, hs], esb[:, hs], mk[mid])
                    elif mid is not None:
                        nc.vector.tensor_mul(esb[:, hs], esb[:, hs], mk[mid])
                    es_tiles[(h, t)] = (esb, hs)

        def pv(cx, h):
            c, pairs, cs, es_tiles, pso_banks = cx
            et, r0 = h // 2, (h % 2) * 64
            if h % 2 == 0:
                pso_banks[h // 2] = psO.tile(
                    [DH + 1, 512], F32, tag="pso", bufs=1, name="pso"
                )
            pso = pso_banks[h // 2][:, (h % 2) * CH : (h % 2 + 1) * CH]
            for i, (t, mid) in enumerate(pairs):
                esb, hs = es_tiles[(h, t)]
                nc.tensor.matmul(
                    pso,
                    lhsT=vA[t][:, h * (DH + 1) : (h + 1) * (DH + 1)],
                    rhs=esb[:, hs],
                    start=(i == 0),
                    stop=(i == len(pairs) - 1),
                    skip_group_check=True,
                )
            rec = awork.tile([1, CH], F32, tag="rec", bufs=3, name="rec")
            nc.vector.reciprocal(rec, pso[DH : DH + 1, :])
            rb = awork.tile([64, CH], F32, tag="rb", bufs=3, name="rb")
            nc.gpsimd.partition_broadcast(rb, rec)
            nc.vector.tensor_mul(aT[et][r0 : r0 + 64, cs], pso[0:DH, :], rb)

        def mkcx(c):
            return (c, chunk_pairs(c), slice(c * CH, (c + 1) * CH), {}, {})

        scale_transpose(0)
        scale_transpose(1)
        for c in range(NCH):
            if c + 1 < NCH:
                scale_transpose(2 * (c + 1))
                scale_transpose(2 * (c + 1) + 1)
            cx = mkcx(c)
            scores(cx, 0)
            scores(cx, 1)
            pv(cx, 0)
            scores(cx, 2)
            pv(cx, 1)
            scores(cx, 3)
            pv(cx, 2)
            pv(cx, 3)
            if c >= 1:
                outproj(c - 1)
        outproj(NCH - 1)

    nc.compile()
    return nc


_NC_CACHE = {}


def get_nc(with_bias: bool):
    if with_bias not in _NC_CACHE:
        _NC_CACHE[with_bias] = _build(with_bias)
    return _NC_CACHE[with_bias]


def _fold_tables(cosf, sinf, w):
    """Fold rmsnorm weight w (per channel) into interleaved-rope cos/sin tables.

    Kernel computes raw*tc + swap(raw)*ts with swap pairing (odd->even,
    even->odd), so:
      tc[:, e]    = cos[:, e]    * w[e]
      ts[:, 2i]   = -sin[:, 2i]  * w[2i+1]
      ts[:, 2i+1] =  sin[:, 2i+1]* w[2i]
    """
    tc_ = cosf * w[None, :]
    ts_ = np.empty_like(sinf)
    ts_[:, 0::2] = -sinf[:, 0::2] * w[None, 1::2]
    ts_[:, 1::2] = sinf[:, 1::2] * w[None, 0::2]
    return tc_, ts_


def _build_masks():
    p = np.arange(128)[:, None]
    j = np.arange(CH)[None, :]
    m = np.empty((6, 128, CH), np.float32)
    m[0] = j >= p
    m[1] = j >= p + 128
    m[2] = j < p + 128
    m[3] = (j < p) | (p == 0)
    m[4] = j < p
    m[5] = p == 0
    return m.astype(ml_dtypes.bfloat16)


def make_in_maps(inputs):
    f = lambda k: np.asarray(inputs[k], np.float32)
    x = f("x")
    cosf = f("cos")[0]
    sinf = f("sin")[0]
    lls = f("logit_log_scale")[0, :, 0]
    bq, bk, bv = f("bq"), f("bk"), f("bv")
    with_bias = bool(np.any(bq) or np.any(bk) or np.any(bv))

    qtc_f, qts_f = _fold_tables(cosf, sinf, f("qn_w"))
    ktc_f, kts_f = _fold_tables(cosf, sinf, f("kn_w"))
    qs = (lls * (1.0 / np.sqrt(DH)))[:, None].astype(np.float32)
    tabs_full = np.stack(
        [qtc_f * qs, qts_f * qs, ktc_f, kts_f], axis=1
    )  # [L, 4, INNER]

    Wq, Wk, Wv, Wo = f("Wq"), f("Wk"), f("Wv"), f("Wo")
    msk = _build_masks()

    xT_b = [np.ascontiguousarray(x[b].T).astype(ml_dtypes.bfloat16) for b in range(B)]
    per_g = []
    for g in range(NG):
        sl = slice(g * EG, (g + 1) * EG)
        gm = {
            "wqT": np.ascontiguousarray(Wq[sl].T).astype(ml_dtypes.bfloat16),
            "wkT": np.ascontiguousarray(Wk[sl].T).astype(ml_dtypes.bfloat16),
            "wvT": np.ascontiguousarray(Wv[sl].T).astype(ml_dtypes.bfloat16),
            "woT": np.ascontiguousarray(Wo[:, sl].T).astype(ml_dtypes.bfloat16),
            "tabs": np.ascontiguousarray(tabs_full[:, :, sl]).astype(ml_dtypes.bfloat16),
            "msk": msk,
        }
        if with_bias:
            gm["bqr"] = bq[None, sl].astype(ml_dtypes.bfloat16)
            gm["bkr"] = bk[None, sl].astype(ml_dtypes.bfloat16)
            gm["bvr"] = bv[None, sl].astype(ml_dtypes.bfloat16)
        per_g.append(gm)

    ims = []
    for c in range(NCORES):
        b, g = divmod(c, NG)
        ims.append({"xT": xT_b[b], **per_g[g]})
    return ims, with_bias


last_results = None


def kernel(**inputs):
    global last_results
    ims, with_bias = make_in_maps(inputs)
    nc = get_nc(with_bias)
    res = run_bass_kernel_spmd(nc, ims, core_ids=list(range(NCORES)))
    last_results = res
    out = np.zeros((B, L, D), np.float32)
    for c, om in enumerate(res.results):
        out[c // NG] += np.asarray(om["outp"], dtype=np.float32)
    out += np.asarray(inputs["bo"], np.float32)[None, None, :]
    return out
